# revision 1
# baseline (speedup 1.0000x reference)
"""Trainium2 Bass kernel for nn_DOZSL_Random (retrieval_knn).

Reference computation (B=256 queries, N=100000 entities, K=4 factors, D=256):
    x = tanh(init_embed @ pca_w + pca_b).reshape(N, K, D)     # entity encoder
    obj_b = x[sub_b, rel_b, :] + init_rel[rel_b]              # query vectors
    score[b, n] = gamma - ||obj_b - x[n, rel_b, :]||^2        # L2 score, factor-selected
    out = sigmoid(score)                                      # [B, N]

Distribution: entity axis N sharded over 8 cores (12500 rows each); queries
replicated. Each core runs the same program (SPMD) on its slab.

Per-core device program (everything heavy stays on device):
  1. encoder: xT[kd, n] = tanh(W^T E^T + b) via PE matmuls (bf16), tanh+bias
     fused on the ACT engine, output kept transposed (d on partitions) so it
     can feed the score GEMM directly as the moving operand.
  2. xsq = xT*xT on the vector engine (bf16).
  3. score GEMM: queries sorted by rel on the host and grouped; for group k
       sel[b, n] = qc[b] + sum_d 2*obj[b,d]*x[n,k,d] - sum_d x[n,k,d]^2
     The -||x||^2 term is folded into the same PSUM accumulation by streaming
     xsq with an all-(-1) stationary operand; qc[b] = gamma - ||obj_b||^2 is
     applied as the per-partition ACT bias of the final fused sigmoid.
  4. sigmoid(psum + qc) on ACT -> fp32 -> DMA to DRAM.

Host does only O(B*D) query prep, transpose/shard/cast, and row un-permutation.
"""

import os
import sys

import numpy as np

for _p in ("/root/.axon_site/_ro/trn_rl_repo", "/opt/trn_rl_repo"):
    if os.path.isdir(_p) and _p not in sys.path:
        sys.path.append(_p)

from contextlib import ExitStack

from concourse import bacc, bass, mybir, tile
from concourse.bass_utils import run_bass_kernel_spmd

dt = mybir.dt

N_CORES = 8
P = 128          # SBUF partitions
MACRO = 1536     # n-columns processed per macro-tile (psum width, 3 banks)
MM_N = 512       # moving-operand width per matmul


def _np_bf16():
    return mybir.dt.np(dt.bfloat16)


def _plan_tiles(group_sizes):
    """Pack rel-groups (in sorted order) into <=128-row psum tiles.

    Returns list of tiles; each tile is a list of segments
    (k, q_lo, q_hi, local_off) with local_off 32-aligned so matmul PSUM
    partition offsets stay 32-aligned. Groups larger than 128 are split.
    """
    segs = []
    q = 0
    for k, s in enumerate(group_sizes):
        s = int(s)
        while s > 0:
            take = min(s, P)
            segs.append((k, q, q + take))
            q += take
            s -= take
    tiles = []
    cur, off = [], 0
    for k, lo, hi in segs:
        rows = hi - lo
        aligned = (off + 31) // 32 * 32
        if aligned + rows > P:
            tiles.append(cur)
            cur, aligned = [], 0
        cur.append((k, lo, hi, aligned))
        off = aligned + rows
    if cur:
        tiles.append(cur)
    return tiles


def _build_program(n_cols, B, init_dim, kd, plan, n_groups):
    """Build the SPMD Bass program for one core's [n_cols] entity slab."""
    nc = bacc.Bacc(
        "TRN2", target_bir_lowering=False, debug=False, enable_asserts=False,
        num_devices=N_CORES,
    )
    ic = init_dim // P          # contraction chunks (2)
    nch = kd // P               # encoder output chunks (8)
    n_tiles = len(plan)

    et_d = nc.dram_tensor("et", [ic, P, n_cols], dt.bfloat16, kind="ExternalInput").ap()
    w_d = nc.dram_tensor("wmat", [ic, P, kd], dt.bfloat16, kind="ExternalInput").ap()
    q_d = nc.dram_tensor("q2t", [ic, P, B], dt.bfloat16, kind="ExternalInput").ap()
    bias_d = nc.dram_tensor("biasc", [nch, P, 1], dt.float32, kind="ExternalInput").ap()
    qc_d = nc.dram_tensor("qcp", [n_tiles, P, 1], dt.float32, kind="ExternalInput").ap()
    out_d = nc.dram_tensor("out", [B, n_cols], dt.float32, kind="ExternalOutput").ap()

    macros = []
    lo = 0
    while lo < n_cols:
        w = min(MACRO, n_cols - lo)
        macros.append((lo, w))
        lo += w

    with tile.TileContext(nc) as tc, ExitStack() as ctx:
        cpool = ctx.enter_context(tc.tile_pool(name="consts", bufs=1))
        w_sb, q_sb = [], []
        for i in range(ic):
            wt = cpool.tile([P, kd], dt.bfloat16, tag=f"w{i}", name=f"w{i}")
            nc.sync.dma_start(out=wt[:], in_=w_d[i])
            w_sb.append(wt)
            qt = cpool.tile([P, B], dt.bfloat16, tag=f"q{i}", name=f"q{i}")
            nc.sync.dma_start(out=qt[:], in_=q_d[i])
            q_sb.append(qt)
        bias_sb = []
        for c in range(nch):
            bt = cpool.tile([P, 1], dt.float32, tag=f"b{c}", name=f"b{c}")
            nc.sync.dma_start(out=bt[:], in_=bias_d[c])
            bias_sb.append(bt)
        qc_sb = []
        for t in range(n_tiles):
            qt = cpool.tile([P, 1], dt.float32, tag=f"qc{t}", name=f"qc{t}")
            nc.sync.dma_start(out=qt[:], in_=qc_d[t])
            qc_sb.append(qt)
        neg1 = cpool.tile([P, P], dt.bfloat16, tag="neg1", name="neg1")
        nc.gpsimd.memset(neg1[:], -1.0)

        et_pool = ctx.enter_context(tc.tile_pool(name="et", bufs=2))
        xt_pool = ctx.enter_context(tc.tile_pool(name="xt", bufs=2))
        xq_pool = ctx.enter_context(tc.tile_pool(name="xq", bufs=2))
        ps_pool = ctx.enter_context(tc.tile_pool(name="ps", bufs=2, space="PSUM"))
        sel_pool = ctx.enter_context(tc.tile_pool(name="sel", bufs=2))

        for lo, w in macros:
            et = []
            for i in range(ic):
                t_ = et_pool.tile([P, w], dt.bfloat16, tag=f"et{i}", name=f"et{i}")
                nc.sync.dma_start(out=t_[:], in_=et_d[i, :, lo:lo + w])
                et.append(t_)

            xts, xqs = [], []
            for c in range(nch):
                ps = ps_pool.tile([P, w], dt.float32, tag="ps", name=f"pse{c}")
                for h0 in range(0, w, MM_N):
                    cw = min(MM_N, w - h0)
                    for i in range(ic):
                        nc.tensor.matmul(
                            ps[:, h0:h0 + cw],
                            lhsT=w_sb[i][:, c * P:(c + 1) * P],
                            rhs=et[i][:, h0:h0 + cw],
                            start=(i == 0), stop=(i == ic - 1),
                        )
                xt = xt_pool.tile([P, w], dt.bfloat16, tag=f"xt{c}", name=f"xt{c}")
                nc.scalar.activation(
                    xt[:], ps[:], mybir.ActivationFunctionType.Tanh,
                    bias=bias_sb[c][:],
                )
                xq = xq_pool.tile([P, w], dt.bfloat16, tag=f"xq{c}", name=f"xq{c}")
                nc.vector.tensor_mul(xq[:], xt[:], xt[:])
                xts.append(xt)
                xqs.append(xq)

            for t, segs in enumerate(plan):
                ps2 = ps_pool.tile([P, w], dt.float32, tag="ps", name=f"pss{t}")
                for (k, qlo, qhi, loff) in segs:
                    rows = qhi - qlo
                    for h0 in range(0, w, MM_N):
                        cw = min(MM_N, w - h0)
                        srcs = [(q_sb[i][:, qlo:qhi], xts[ic * k + i]) for i in range(ic)]
                        srcs += [(neg1[:, :rows], xqs[ic * k + i]) for i in range(ic)]
                        for si, (lhsT, xsrc) in enumerate(srcs):
                            nc.tensor.matmul(
                                ps2[loff:loff + rows, h0:h0 + cw],
                                lhsT=lhsT,
                                rhs=xsrc[:, h0:h0 + cw],
                                start=(si == 0), stop=(si == len(srcs) - 1),
                            )
                sel = sel_pool.tile([P, w], dt.float32, tag=f"sel{t}", name=f"sel{t}")
                for (k, qlo, qhi, loff) in segs:
                    rows = qhi - qlo
                    nc.scalar.activation(
                        sel[loff:loff + rows, :], ps2[loff:loff + rows, :],
                        mybir.ActivationFunctionType.Sigmoid,
                        bias=qc_sb[t][loff:loff + rows, :],
                    )
                for (k, qlo, qhi, loff) in segs:
                    rows = qhi - qlo
                    nc.sync.dma_start(
                        out=out_d[qlo:qhi, lo:lo + w],
                        in_=sel[loff:loff + rows, :],
                    )

    nc.compile()
    return nc


def _host_prep(sub, rel, init_embed, init_rel, pca_w, pca_b, gamma):
    """All O(B*D + reshaping) host-side preparation. Returns (nc, in_maps, meta)."""
    bf16 = _np_bf16()
    N, init_dim = init_embed.shape
    D = init_rel.shape[1]
    kd = pca_w.shape[1]
    K = kd // D
    B = sub.shape[0]
    assert N % N_CORES == 0
    n_cols = N // N_CORES
    ic = init_dim // P
    nch = kd // P

    # ---- query-side prep (tiny: B rows) -------------------------------
    e_sub = init_embed[np.asarray(sub)]                       # [B, init_dim]
    x_sub = np.tanh(e_sub @ pca_w + pca_b).reshape(B, K, D)
    relv = np.asarray(rel).astype(np.int64)
    sub_sel = x_sub[np.arange(B), relv]                       # [B, D]
    obj = sub_sel + init_rel[relv]                            # [B, D]
    qc = (float(gamma[0]) - (obj * obj).sum(-1)).astype(np.float32)   # [B]

    perm = np.argsort(relv, kind="stable")
    group_sizes = np.bincount(relv, minlength=K)
    plan = _plan_tiles(group_sizes)

    q2 = (2.0 * obj[perm]).astype(np.float32)                 # [B, D] sorted
    q2t = np.ascontiguousarray(q2.T).reshape(ic, P, B).astype(bf16)

    qc_sorted = qc[perm]
    qcp = np.zeros((len(plan), P, 1), dtype=np.float32)
    for t, segs in enumerate(plan):
        for (k, qlo, qhi, loff) in segs:
            qcp[t, loff:loff + (qhi - qlo), 0] = qc_sorted[qlo:qhi]

    w_chunks = np.ascontiguousarray(pca_w).reshape(ic, P, kd).astype(bf16)
    bias_c = np.ascontiguousarray(pca_b).astype(np.float32).reshape(nch, P, 1)

    et_full = np.ascontiguousarray(init_embed.T).astype(bf16)  # [init_dim, N]
    et_full = et_full.reshape(ic, P, N)

    in_maps = []
    for c in range(N_CORES):
        in_maps.append({
            "et": np.ascontiguousarray(et_full[:, :, c * n_cols:(c + 1) * n_cols]),
            "wmat": w_chunks,
            "q2t": q2t,
            "biasc": bias_c,
            "qcp": qcp,
        })

    nc = _build_program(n_cols, B, init_dim, kd, plan, K)
    meta = dict(perm=perm, B=B, N=N, n_cols=n_cols)
    return nc, in_maps, meta


def _assemble(results, meta):
    stacked = np.concatenate([results[c]["out"] for c in range(N_CORES)], axis=1)
    out = np.empty((meta["B"], meta["N"]), dtype=np.float32)
    out[meta["perm"]] = stacked
    return out


def kernel(sub, rel, init_embed, init_rel, pca_w, pca_b, gamma):
    sub = np.asarray(sub)
    rel = np.asarray(rel)
    init_embed = np.asarray(init_embed, dtype=np.float32)
    init_rel = np.asarray(init_rel, dtype=np.float32)
    pca_w = np.asarray(pca_w, dtype=np.float32)
    pca_b = np.asarray(pca_b, dtype=np.float32)
    gamma = np.asarray(gamma, dtype=np.float32)

    nc, in_maps, meta = _host_prep(
        sub, rel, init_embed, init_rel, pca_w, pca_b, gamma
    )
    res = run_bass_kernel_spmd(nc, in_maps, list(range(N_CORES)))
    return _assemble(res.results, meta)
